# revision 1
# baseline (speedup 1.0000x reference)
"""Trainium2 Bass kernel for nn_CT_37821482009425 (snntorch Leaky LIF scan).

Reference semantics (bitwise-matched):
    T = clip(t, 1, 5); x = roll(inp, roll_amount, axis=2)
    per step: reset = (mem > T); mem = 0.95*mem + x_t - reset*T; spk = (mem > T)
Output: spikes (1024, 1, 224, 224) float32 in {0, 1}.

Distribution: pure data parallelism — batch 1024 -> 8 cores x 128 (the SBUF
partition dim). Host prep per core: apply the roll and transpose to
time-major so each timestep's H=224 vector is contiguous per partition.

Per-core compute (vector engine, per time step, all stock DVE ops whose
rounding matches the reference exactly):
    v      = scalar_tensor_tensor(mem[t-1], 0.95, x_t, mult, add)
    mem_t  = tensor_tensor(v, r[t-1], subtract)
    r_t    = tensor_scalar(mem_t, T, T, is_gt, mult)        # in {0, T}
r_t doubles as the reset feedback and the spike record (spk = r != 0 on host).
Input DMA (sync engine) and output DMA (scalar engine) run in 8-step slices,
double-buffered against compute in 32-step chunk buffers.
"""

import numpy as np
import concourse.bass as bass
import concourse.mybir as mybir
from concourse.bass_utils import run_bass_kernel_spmd

BETA = 0.95
B, CH = 1024, 224
N_CORES = 8
PB = B // N_CORES  # 128 batches per core = partition dim
H = CH  # per-step vector length (contiguous, time-major)
W = CH  # time steps
WC = 32  # chunk size (SBUF buffer granularity)
SUB = 2  # DMA slice granularity (steps)
N_CHUNK = W // WC
SUBS_PER_CHUNK = WC // SUB
N_SUB = W // SUB

_Alu = mybir.AluOpType

_cache = {}


def _build(T: float):
    nc = bass.Bass(trn_type="TRN2")
    x_d = nc.dram_tensor("x", [PB, W * H], mybir.dt.float32, kind="ExternalInput")
    r_d = nc.dram_tensor("r", [PB, W * H], mybir.dt.float32, kind="ExternalOutput")

    with (
        nc.sbuf_tensor("xt0", [PB, WC * H], mybir.dt.float32) as xt0,
        nc.sbuf_tensor("xt1", [PB, WC * H], mybir.dt.float32) as xt1,
        nc.sbuf_tensor("rt0", [PB, WC * H], mybir.dt.float32) as rt0,
        nc.sbuf_tensor("rt1", [PB, WC * H], mybir.dt.float32) as rt1,
        nc.sbuf_tensor("mcol", [PB, 2 * H], mybir.dt.float32) as mcol,
        nc.sbuf_tensor("vcol", [PB, H], mybir.dt.float32) as vcol,
        nc.semaphore() as in_sem,
        nc.semaphore() as v_sem,
        nc.semaphore() as out_sem,
        nc.Block() as block,
    ):
        xb = [xt0, xt1]
        rb = [rt0, rt1]

        # v_sem: vector increments once per completed SUB-slice (28 total).

        @block.sync
        def _(sync):
            # input DMA, one 8-step slice at a time
            for sb in range(N_SUB):
                c, sl = divmod(sb, SUBS_PER_CHUNK)
                if c >= 2:
                    # xt[c%2] slice sl is free once chunk c-2's compute has
                    # fully consumed that slice (vector bumps v_sem per slice)
                    sync.wait_ge(v_sem, (c - 2) * SUBS_PER_CHUNK + sl + 1)
                sync.dma_start(
                    xb[c % 2][:, sl * SUB * H : (sl + 1) * SUB * H],
                    x_d[:, sb * SUB * H : (sb + 1) * SUB * H],
                ).then_inc(in_sem, 16)

        @block.scalar
        def _(scalar):
            for sb in range(N_SUB):
                c, sl = divmod(sb, SUBS_PER_CHUNK)
                scalar.wait_ge(v_sem, sb + 1)
                scalar.dma_start(
                    r_d[:, sb * SUB * H : (sb + 1) * SUB * H],
                    rb[c % 2][:, sl * SUB * H : (sl + 1) * SUB * H],
                ).then_inc(out_sem, 16)

        @block.vector
        def _(vector):
            for sb in range(N_SUB):
                c, sl = divmod(sb, SUBS_PER_CHUNK)
                vector.wait_ge(in_sem, 16 * (sb + 1))
                if sl == 0 and c >= 2:
                    # rt[c%2] free once all its out-DMA slices (chunk c-2)
                    # completed
                    vector.wait_ge(out_sem, 16 * (c - 1) * SUBS_PER_CHUNK)
                xt, rt = xb[c % 2], rb[c % 2]
                for tl in range(sl * SUB, (sl + 1) * SUB):
                    t = c * WC + tl
                    xcol = xt[:, tl * H : (tl + 1) * H]
                    rcol = rt[:, tl * H : (tl + 1) * H]
                    mc = mcol[:, (t % 2) * H : (t % 2 + 1) * H]
                    if t == 0:
                        nc.vector.tensor_copy(mc, xcol)
                    else:
                        mp = mcol[:, ((t - 1) % 2) * H : ((t - 1) % 2 + 1) * H]
                        if tl == 0:
                            rprev = rb[(c - 1) % 2][:, (WC - 1) * H :]
                        else:
                            rprev = rt[:, (tl - 1) * H : tl * H]
                        nc.vector.scalar_tensor_tensor(
                            vcol[:], mp, BETA, xcol, _Alu.mult, _Alu.add
                        )
                        nc.vector.tensor_tensor(mc, vcol[:], rprev, _Alu.subtract)
                    ts = nc.vector.tensor_scalar(
                        rcol, mc, T, T, _Alu.is_gt, _Alu.mult
                    )
                    if tl % SUB == SUB - 1:
                        ts.then_inc(v_sem, 1)

    return nc


def kernel(inp: np.ndarray, t: np.ndarray, roll_amount) -> np.ndarray:
    T = float(
        np.clip(np.float32(np.asarray(t).reshape(-1)[0]), np.float32(1.0),
                np.float32(5.0))
    )
    roll = int(np.asarray(roll_amount)) % W

    key = (T,)
    if key not in _cache:
        _cache[key] = _build(T)
    nc = _cache[key]

    inp = np.asarray(inp, dtype=np.float32).reshape(B, CH, CH)
    in_maps = []
    for c in range(N_CORES):
        shard = inp[c * PB : (c + 1) * PB]  # (128, H, W)
        shard = np.roll(shard, roll, axis=2)
        # time-major: (128, W, H) contiguous
        x_tm = np.ascontiguousarray(shard.transpose(0, 2, 1)).reshape(PB, W * H)
        in_maps.append({"x": x_tm})

    res = run_bass_kernel_spmd(nc, in_maps, core_ids=list(range(N_CORES)))

    out = np.empty((B, 1, CH, CH), dtype=np.float32)
    for c in range(N_CORES):
        r = res.results[c]["r"].reshape(PB, W, H)  # (b, w, h)
        out[c * PB : (c + 1) * PB, 0] = (r != 0).transpose(0, 2, 1)
    return out



# revision 3
# speedup vs baseline: 1.2486x; 1.2486x over previous
"""Trainium2 Bass kernel for nn_CT_37821482009425 (snntorch Leaky LIF scan).

Reference semantics:
    T = clip(t, 1, 5); x = roll(inp, roll_amount, axis=2)
    per step: reset = (mem > T); mem = 0.95*mem + x_t - reset*T; spk = (mem > T)
Output: spikes (1024, 1, 224, 224) float32 in {0, 1}.

Distribution: pure data parallelism - batch 1024 -> 8 cores x 128 partitions.
Host prep per core: roll + transpose to time-major, then affine-rescale the
input so the whole recurrence collapses to 2 ALU ops per step with the
threshold at zero:

    s_t = mem_t*(BETA/T) - BETA      (threshold crossing ⇔ s_t > 0)
    c_t = s_t + (s_t <= 0)           (carry; folds the reset subtract)
    s_t = BETA*c_{t-1} + y_t,   y_t = x_t*(BETA/T) + BETA^2 - 2*BETA  (host)
    init: c = 0.05  (== mem0 = 0)

Device per step per engine population (H=224 neurons split DVE | Pool):
    op1: s = scalar_tensor_tensor(c, BETA, y_t, mult, add)
    op2: c = scalar_tensor_tensor(s, 0.0, s, is_le, add)
Spike extraction runs OFF the critical path on the Act engine in 8-step
chunks: spk_u8 = Sign(s) (uint8; host decodes spk = (v == 1)), so the
output DMA is 1 byte per element instead of 4.

The kernel binary is independent of T and roll (both folded into host prep),
so a single compiled module is reused for all calls.
"""

import numpy as np
import concourse.bass as bass
import concourse.mybir as mybir
from concourse.bass_utils import run_bass_kernel_spmd

BETA = 0.95
B, CH = 1024, 224
N_CORES = 8
PB = B // N_CORES  # 128 batches per core = partition dim
H = CH  # neurons per partition (free dim per step)
W = CH  # time steps
ND = 224  # neurons handled by DVE
NP = H - ND  # neurons handled by Pool (gpsimd)
CHUNK = 8  # steps per DMA slice / extraction chunk
RINGC = 4  # ring depth in chunks (input, state, and spike rings)
RING = RINGC * CHUNK  # ring depth in steps
N_CHUNK = W // CHUNK

_Alu = mybir.AluOpType

_cache = {}


def _build():
    nc = bass.Bass(trn_type="TRN2")
    y_d = nc.dram_tensor("y", [PB, W * H], mybir.dt.float32, kind="ExternalInput")
    r_d = nc.dram_tensor("r", [PB, W * H], mybir.dt.uint8, kind="ExternalOutput")

    with (
        nc.sbuf_tensor("yring", [PB, RING * H], mybir.dt.float32) as yring,
        nc.sbuf_tensor("sring", [PB, RING * H], mybir.dt.float32) as sring,
        nc.sbuf_tensor("kring", [PB, RING * H], mybir.dt.uint8) as kring,
        nc.sbuf_tensor("cbuf", [PB, H], mybir.dt.float32) as cbuf,
        nc.semaphore() as in_sem,
        nc.semaphore() as dve_sem,
        nc.semaphore() as pool_sem,
        nc.semaphore() as act_sem,
        nc.semaphore() as out_sem,
        nc.Block() as block,
    ):
        CB = CHUNK * H  # elements per chunk per partition

        @block.sync
        def _(sync):
            def dma_in(c):
                if c >= RINGC:
                    # y-ring slot free once chunk c-RINGC fully consumed
                    sync.wait_ge(dve_sem, c - RINGC + 1)
                    if NP:
                        sync.wait_ge(pool_sem, c - RINGC + 1)
                sync.dma_start(
                    yring[:, (c % RINGC) * CB : (c % RINGC + 1) * CB],
                    y_d[:, c * CB : (c + 1) * CB],
                ).then_inc(in_sem, 16)

            for c in range(min(RINGC, N_CHUNK)):
                dma_in(c)
            for c in range(N_CHUNK):
                sync.wait_ge(act_sem, c + 1)
                sync.dma_start(
                    r_d[:, c * CB : (c + 1) * CB],
                    kring[:, (c % RINGC) * CB : (c % RINGC + 1) * CB],
                ).then_inc(out_sem, 16)
                if c + RINGC < N_CHUNK:
                    dma_in(c + RINGC)

        def compute(eng, off, width, sem):
            eng.memset(cbuf[:, off : off + width], 0.05)
            for c in range(N_CHUNK):
                eng.wait_ge(in_sem, 16 * (c + 1))
                if c >= RINGC:
                    # s-ring slot free once chunk c-RINGC extracted by Act
                    eng.wait_ge(act_sem, c - RINGC + 1)
                for tl in range(CHUNK):
                    t = c * CHUNK + tl
                    sl = t % RING
                    scol = sring[:, sl * H + off : sl * H + off + width]
                    ycol = yring[:, sl * H + off : sl * H + off + width]
                    ccol = cbuf[:, off : off + width]
                    eng.scalar_tensor_tensor(
                        scol, ccol, BETA, ycol, _Alu.mult, _Alu.add
                    )
                    ts = eng.scalar_tensor_tensor(
                        ccol, scol, 0.0, scol, _Alu.is_le, _Alu.add
                    )
                    if tl == CHUNK - 1:
                        ts.then_inc(sem, 1)

        @block.vector
        def _(vector):
            compute(nc.vector, 0, ND, dve_sem)

        if NP:

            @block.gpsimd
            def _(gpsimd):
                compute(nc.gpsimd, ND, NP, pool_sem)

        @block.scalar
        def _(scalar):
            for c in range(N_CHUNK):
                scalar.wait_ge(dve_sem, c + 1)
                if NP:
                    scalar.wait_ge(pool_sem, c + 1)
                if c >= RINGC:
                    # spike-ring slot free once chunk c-RINGC DMA'd out
                    scalar.wait_ge(out_sem, 16 * (c - RINGC + 1))
                nc.scalar.activation(
                    kring[:, (c % RINGC) * CB : (c % RINGC + 1) * CB],
                    sring[:, (c % RINGC) * CB : (c % RINGC + 1) * CB],
                    mybir.ActivationFunctionType.Sign,
                    0.0,
                    1.0,
                ).then_inc(act_sem, 1)

    return nc


def kernel(inp: np.ndarray, t: np.ndarray, roll_amount) -> np.ndarray:
    T = float(
        np.clip(np.float32(np.asarray(t).reshape(-1)[0]), np.float32(1.0),
                np.float32(5.0))
    )
    roll = int(np.asarray(roll_amount)) % W

    if "k" not in _cache:
        _cache["k"] = _build()
    nc = _cache["k"]

    scale = np.float64(0.95) / np.float64(np.float32(T))
    K = np.float64(0.95) * np.float64(0.95) - np.float64(1.9)

    inp = np.asarray(inp, dtype=np.float32).reshape(B, CH, CH)
    x = np.roll(inp, roll, axis=2)  # (B, H, W)
    x = np.ascontiguousarray(x.transpose(0, 2, 1))  # (B, W, H) time-major
    y = (x.astype(np.float64) * scale + K).astype(np.float32)

    in_maps = [
        {"y": y[c * PB : (c + 1) * PB].reshape(PB, W * H)} for c in range(N_CORES)
    ]
    res = run_bass_kernel_spmd(nc, in_maps, core_ids=list(range(N_CORES)))

    out = np.empty((B, 1, CH, CH), dtype=np.float32)
    for c in range(N_CORES):
        r = res.results[c]["r"].reshape(PB, W, H)  # (b, w, h) uint8
        out[c * PB : (c + 1) * PB, 0] = (r == 1).transpose(0, 2, 1)
    return out


# revision 4
# speedup vs baseline: 1.4295x; 1.1449x over previous
"""Trainium2 Bass kernel for nn_CT_37821482009425 (snntorch Leaky LIF scan).

Reference semantics:
    T = clip(t, 1, 5); x = roll(inp, roll_amount, axis=2)
    per step: reset = (mem > T); mem = 0.95*mem + x_t - reset*T; spk = (mem > T)
Output: spikes (1024, 1, 224, 224) float32 in {0, 1}.

Distribution: pure data parallelism - batch 1024 -> 8 cores x 128 partitions.
Host prep per core: roll + transpose to time-major, then affine-rescale the
input so the threshold sits at zero and the reset quantum becomes exactly 1:

    s_t = mem_t*(BETA/T) - BETA      (spike ⇔ s_t > 0)
    s_t = BETA*s_{t-1} - BETA*spk_{t-1} + y_t,  y_t = x_t*(BETA/T) + BETA^2-BETA

The H=224 neurons per partition are split across two engines that each run
an independent serial scan (no cross-engine deps in the recurrence):

DVE (h < ND), carry form, 2 scalar_tensor_tensor ops/step:
    op1: s = (c * BETA) + ytilde          ytilde = y - BETA  (host folds)
    op2: c = (s <= 0) + s                 (c = s - spk + 1)

Pool/gpsimd (h >= ND) cannot run STT; it runs the time-rescaled sigma form
(sigma = s * BETA^(-tau), tau = t mod RN) with 3 TS/TT ops per step and a
renorm multiply every RN steps:
    [tau==0, t>0]  sig = sig * BETA^RN
    d   = (sig > 0) * (-BETA^(1-tau))
    u   = sig + d
    sig = u + w_t                         w_t = y_t * BETA^(-tau)  (host)

Spike extraction runs OFF the critical path on the Act engine in CHUNK-step
blocks: spk_u8 = Sign(state) (uint8; host decodes spk = (v == 1)), so the
output DMA is 1 byte per element instead of 4.

The kernel binary is independent of T and roll (both folded into host prep),
so a single compiled module is reused for all calls.
"""

import numpy as np
import concourse.bass as bass
import concourse.mybir as mybir
from concourse.bass_utils import run_bass_kernel_spmd

BETA = 0.95
B, CH = 1024, 224
N_CORES = 8
PB = B // N_CORES  # 128 batches per core = partition dim
H = CH  # neurons per partition (free dim per step)
W = CH  # time steps
ND = 184  # neurons handled by DVE
NP = H - ND  # neurons handled by Pool (gpsimd)
RN = 16  # Pool sigma-form renorm period (steps)
CHUNK = 8  # steps per DMA slice / extraction chunk
RINGC = 4  # ring depth in chunks (input, state, and spike rings)
RING = RINGC * CHUNK  # ring depth in steps
N_CHUNK = W // CHUNK

_Alu = mybir.AluOpType

_F32B = np.float32(BETA)
_SIG_INIT = float(-(_F32B * _F32B))  # sigma_{-1} == s_{-1} = -BETA, pre-decayed
_RENORM = float(np.float32(np.float64(BETA) ** RN))
_DC = [float(-np.float32(np.float64(BETA) ** (1 - tau))) for tau in range(RN)]

_cache = {}


def _build():
    nc = bass.Bass(trn_type="TRN2")
    y_d = nc.dram_tensor("y", [PB, W * H], mybir.dt.float32, kind="ExternalInput")
    r_d = nc.dram_tensor("r", [PB, W * H], mybir.dt.uint8, kind="ExternalOutput")

    with (
        nc.sbuf_tensor("yring", [PB, RING * H], mybir.dt.float32) as yring,
        nc.sbuf_tensor("sring", [PB, RING * H], mybir.dt.float32) as sring,
        nc.sbuf_tensor("kring", [PB, RING * H], mybir.dt.uint8) as kring,
        nc.sbuf_tensor("cbuf", [PB, max(ND, 1)], mybir.dt.float32) as cbuf,
        nc.sbuf_tensor("pd0", [PB, max(NP, 1)], mybir.dt.float32) as pd0,
        nc.sbuf_tensor("pd1", [PB, max(NP, 1)], mybir.dt.float32) as pd1,
        nc.sbuf_tensor("pd2", [PB, max(NP, 1)], mybir.dt.float32) as pd2,
        nc.semaphore() as in_sem,
        nc.semaphore() as dve_sem,
        nc.semaphore() as pool_sem,
        nc.semaphore() as act_sem,
        nc.semaphore() as out_sem,
        nc.Block() as block,
    ):
        CB = CHUNK * H  # elements per chunk per partition

        @block.sync
        def _(sync):
            def dma_in(c):
                if c >= RINGC:
                    # y-ring slot free once chunk c-RINGC fully consumed
                    sync.wait_ge(dve_sem, c - RINGC + 1)
                    if NP:
                        sync.wait_ge(pool_sem, c - RINGC + 1)
                sync.dma_start(
                    yring[:, (c % RINGC) * CB : (c % RINGC + 1) * CB],
                    y_d[:, c * CB : (c + 1) * CB],
                ).then_inc(in_sem, 16)

            for c in range(min(RINGC, N_CHUNK)):
                dma_in(c)
            for c in range(N_CHUNK):
                sync.wait_ge(act_sem, c + 1)
                sync.dma_start(
                    r_d[:, c * CB : (c + 1) * CB],
                    kring[:, (c % RINGC) * CB : (c % RINGC + 1) * CB],
                ).then_inc(out_sem, 16)
                if c + RINGC < N_CHUNK:
                    dma_in(c + RINGC)

        @block.vector
        def _(vector):
            eng = nc.vector
            eng.memset(cbuf[:, :ND], 0.05)
            for c in range(N_CHUNK):
                eng.wait_ge(in_sem, 16 * (c + 1))
                if c >= RINGC:
                    # s-ring slot free once chunk c-RINGC extracted by Act
                    eng.wait_ge(act_sem, c - RINGC + 1)
                for tl in range(CHUNK):
                    t = c * CHUNK + tl
                    sl = t % RING
                    scol = sring[:, sl * H : sl * H + ND]
                    ycol = yring[:, sl * H : sl * H + ND]
                    ccol = cbuf[:, :ND]
                    eng.scalar_tensor_tensor(
                        scol, ccol, BETA, ycol, _Alu.mult, _Alu.add
                    )
                    ts = eng.scalar_tensor_tensor(
                        ccol, scol, 0.0, scol, _Alu.is_le, _Alu.add
                    )
                    if tl == CHUNK - 1:
                        ts.then_inc(dve_sem, 1)

        if NP:

            @block.gpsimd
            def _(eng_q):
                eng = nc.gpsimd
                eng.memset(pd0[:, :NP], _SIG_INIT)
                for c in range(N_CHUNK):
                    eng.wait_ge(in_sem, 16 * (c + 1))
                    if c >= RINGC:
                        eng.wait_ge(act_sem, c - RINGC + 1)
                    for tl in range(CHUNK):
                        t = c * CHUNK + tl
                        tau = t % RN
                        sl = t % RING
                        if t == 0:
                            prev = pd0[:, :NP]
                        else:
                            psl = (t - 1) % RING
                            prev = sring[:, psl * H + ND : psl * H + H]
                            if tau == 0:
                                eng.tensor_scalar(
                                    pd0[:, :NP], prev, _RENORM, None,
                                    _Alu.mult, _Alu.bypass,
                                )
                                prev = pd0[:, :NP]
                        wcol = yring[:, sl * H + ND : sl * H + H]
                        ocol = sring[:, sl * H + ND : sl * H + H]
                        eng.tensor_scalar(
                            pd1[:, :NP], prev, 0.0, _DC[tau],
                            _Alu.is_gt, _Alu.mult,
                        )
                        eng.tensor_tensor(pd2[:, :NP], prev, pd1[:, :NP], _Alu.add)
                        tt = eng.tensor_tensor(ocol, pd2[:, :NP], wcol, _Alu.add)
                        if tl == CHUNK - 1:
                            tt.then_inc(pool_sem, 1)

        @block.scalar
        def _(scalar):
            for c in range(N_CHUNK):
                scalar.wait_ge(dve_sem, c + 1)
                if NP:
                    scalar.wait_ge(pool_sem, c + 1)
                if c >= RINGC:
                    # spike-ring slot free once chunk c-RINGC DMA'd out
                    scalar.wait_ge(out_sem, 16 * (c - RINGC + 1))
                nc.scalar.activation(
                    kring[:, (c % RINGC) * CB : (c % RINGC + 1) * CB],
                    sring[:, (c % RINGC) * CB : (c % RINGC + 1) * CB],
                    mybir.ActivationFunctionType.Sign,
                    0.0,
                    1.0,
                ).then_inc(act_sem, 1)

    return nc


def kernel(inp: np.ndarray, t: np.ndarray, roll_amount) -> np.ndarray:
    T = float(
        np.clip(np.float32(np.asarray(t).reshape(-1)[0]), np.float32(1.0),
                np.float32(5.0))
    )
    roll = int(np.asarray(roll_amount)) % W

    if "k" not in _cache:
        _cache["k"] = _build()
    nc = _cache["k"]

    scale = np.float64(0.95) / np.float64(np.float32(T))
    K1 = np.float64(0.95) * np.float64(0.95) - np.float64(0.95)  # s-form const
    K2 = K1 - np.float64(0.95)  # DVE carry-form const (BETA^2 - 2 BETA)

    inp = np.asarray(inp, dtype=np.float32).reshape(B, CH, CH)
    x = np.roll(inp, roll, axis=2)  # (B, H, W)
    x = np.ascontiguousarray(x.transpose(0, 2, 1))  # (B, W, H) time-major
    x64 = x.astype(np.float64)
    y = np.empty((B, W, H), dtype=np.float32)
    y[:, :, :ND] = (x64[:, :, :ND] * scale + K2).astype(np.float32)
    if NP:
        bpow = (np.float64(0.95) ** (-(np.arange(W) % RN)))[None, :, None]
        y[:, :, ND:] = (x64[:, :, ND:] * (scale * bpow) + K1 * bpow).astype(
            np.float32
        )

    in_maps = [
        {"y": y[c * PB : (c + 1) * PB].reshape(PB, W * H)} for c in range(N_CORES)
    ]
    res = run_bass_kernel_spmd(nc, in_maps, core_ids=list(range(N_CORES)))

    out = np.empty((B, 1, CH, CH), dtype=np.float32)
    for c in range(N_CORES):
        r = res.results[c]["r"].reshape(PB, W, H)  # (b, w, h) uint8
        out[c * PB : (c + 1) * PB, 0] = (r == 1).transpose(0, 2, 1)
    return out


# revision 12
# speedup vs baseline: 1.4673x; 1.0264x over previous
"""Trainium2 Bass kernel for nn_CT_37821482009425 (snntorch Leaky LIF scan).

Reference semantics:
    T = clip(t, 1, 5); x = roll(inp, roll_amount, axis=2)
    per step: reset = (mem > T); mem = 0.95*mem + x_t - reset*T; spk = (mem > T)
Output: spikes (1024, 1, 224, 224) float32 in {0, 1}.

Distribution: pure data parallelism - batch 1024 -> 8 cores x 128 partitions.
Host prep per core: roll + transpose to time-major, then affine-rescale the
input so the threshold sits at zero and the reset quantum becomes exactly 1:

    s_t = mem_t*(BETA/T) - BETA      (spike ⇔ s_t > 0)
    s_t = BETA*s_{t-1} - BETA*spk_{t-1} + y_t,  y_t = x_t*(BETA/T) + BETA^2-BETA

The H=224 neurons per partition are split across two engines that each run
an independent serial scan (no cross-engine deps in the recurrence):

DVE (h < ND), carry form, 2 scalar_tensor_tensor ops/step:
    op1: s = (c * BETA) + ytilde          ytilde = y - BETA  (host folds)
    op2: c = (s <= 0) + s                 (c = s - spk + 1)

Pool/gpsimd (h >= ND) cannot run STT; it runs the time-rescaled sigma form
(sigma = s * BETA^(-tau), tau = t mod RN) with 3 TS/TT ops per step and a
renorm multiply every RN steps:
    [tau==0, t>0]  sig = sig * BETA^RN
    d   = (sig > 0) * (-BETA^(1-tau))
    u   = sig + d
    sig = u + w_t                         w_t = y_t * BETA^(-tau)  (host)

Spike extraction runs OFF the critical path on the Act engine in CHUNK-step
blocks: spk_u8 = Sign(state) (uint8; host decodes spk = (v == 1)), so the
output DMA is 1 byte per element instead of 4.

The kernel binary is independent of T and roll (both folded into host prep),
so a single compiled module is reused for all calls.
"""

import numpy as np
import concourse.bass as bass
import concourse.mybir as mybir
from concourse.bass_utils import run_bass_kernel_spmd

BETA = 0.95
B, CH = 1024, 224
N_CORES = 8
PB = B // N_CORES  # 128 batches per core = partition dim
H = CH  # neurons per partition (free dim per step)
W = CH  # time steps
ND = 185  # neurons handled by DVE
NP = H - ND  # neurons handled by Pool (gpsimd)
RN = 16  # Pool sigma-form renorm period (steps)
CHUNK = 8  # steps per DMA slice / extraction chunk
RINGC = 4  # ring depth in chunks (input, state, and spike rings)
RING = RINGC * CHUNK  # ring depth in steps
N_CHUNK = W // CHUNK

_Alu = mybir.AluOpType

_F32B = np.float32(BETA)
_SIG_INIT = float(-(_F32B * _F32B))  # sigma_{-1} == s_{-1} = -BETA, pre-decayed
_RENORM = float(np.float32(np.float64(BETA) ** RN))
_DC = [float(-np.float32(np.float64(BETA) ** (1 - tau))) for tau in range(RN)]

_cache = {}


def _build():
    nc = bass.Bass(trn_type="TRN2")
    y_d = nc.dram_tensor("y", [PB, W * H], mybir.dt.float32, kind="ExternalInput")
    r_d = nc.dram_tensor("r", [PB, W * H], mybir.dt.uint8, kind="ExternalOutput")

    with (
        nc.sbuf_tensor("yring", [PB, RING * H], mybir.dt.float32) as yring,
        nc.sbuf_tensor("sring", [PB, RING * H], mybir.dt.float32) as sring,
        nc.sbuf_tensor("kring", [PB, RING * H], mybir.dt.uint8) as kring,
        nc.sbuf_tensor("cbuf", [PB, max(ND, 1)], mybir.dt.float32) as cbuf,
        nc.sbuf_tensor("pd0", [PB, max(NP, 1)], mybir.dt.float32) as pd0,
        nc.sbuf_tensor("pd1", [PB, max(NP, 1)], mybir.dt.float32) as pd1,
        nc.sbuf_tensor("pd2", [PB, max(NP, 1)], mybir.dt.float32) as pd2,
        nc.semaphore() as in_sem,
        nc.semaphore() as dve_sem,
        nc.semaphore() as pool_sem,
        nc.semaphore() as act_sem,
        nc.semaphore() as out_sem,
        nc.Block() as block,
    ):
        CB = CHUNK * H  # elements per chunk per partition
        LAST = N_CHUNK - 1
        NSUB = CHUNK // 2  # 2-step sub-slices for pipeline fill/drain

        @block.sync
        def _(sync):
            def dma_in(c):
                if c == 0:
                    # fine-grained first chunk so compute starts ~3us earlier
                    for g in range(NSUB):
                        sync.dma_start(
                            yring[:, g * 2 * H : (g + 1) * 2 * H],
                            y_d[:, g * 2 * H : (g + 1) * 2 * H],
                        ).then_inc(in_sem, 16)
                    return
                if c >= RINGC:
                    # y-ring slot free once chunk c-RINGC fully consumed
                    sync.wait_ge(dve_sem, c - RINGC + 1)
                    if NP:
                        sync.wait_ge(pool_sem, c - RINGC + 1)
                sync.dma_start(
                    yring[:, (c % RINGC) * CB : (c % RINGC + 1) * CB],
                    y_d[:, c * CB : (c + 1) * CB],
                ).then_inc(in_sem, 16)

            for c in range(min(RINGC, N_CHUNK)):
                dma_in(c)
            for c in range(N_CHUNK - 1):
                sync.wait_ge(act_sem, c + 1)
                sync.dma_start(
                    r_d[:, c * CB : (c + 1) * CB],
                    kring[:, (c % RINGC) * CB : (c % RINGC + 1) * CB],
                ).then_inc(out_sem, 16)
                if c + RINGC < N_CHUNK:
                    dma_in(c + RINGC)
            # fine-grained last chunk: extraction pieces land every 2 steps
            for g in range(NSUB):
                sync.wait_ge(act_sem, N_CHUNK + g)
                off = LAST * CB + g * 2 * H
                roff = (LAST % RINGC) * CB + g * 2 * H
                sync.dma_start(
                    r_d[:, off : off + 2 * H],
                    kring[:, roff : roff + 2 * H],
                ).then_inc(out_sem, 16)

        @block.vector
        def _(vector):
            eng = nc.vector
            eng.memset(cbuf[:, :ND], 0.05)
            for c in range(N_CHUNK):
                if c > 0:
                    eng.wait_ge(in_sem, 16 * (NSUB + c))
                if c >= RINGC:
                    # s-ring slot free once chunk c-RINGC extracted by Act
                    eng.wait_ge(act_sem, c - RINGC + 1)
                for tl in range(CHUNK):
                    t = c * CHUNK + tl
                    if c == 0 and tl % 2 == 0:
                        eng.wait_ge(in_sem, 16 * (tl // 2 + 1))
                    sl = t % RING
                    scol = sring[:, sl * H : sl * H + ND]
                    ycol = yring[:, sl * H : sl * H + ND]
                    ccol = cbuf[:, :ND]
                    eng.scalar_tensor_tensor(
                        scol, ccol, BETA, ycol, _Alu.mult, _Alu.add
                    )
                    ts = eng.scalar_tensor_tensor(
                        ccol, scol, 0.0, scol, _Alu.is_le, _Alu.add
                    )
                    if (c == LAST and tl % 2 == 1) or (
                        c < LAST and tl == CHUNK - 1
                    ):
                        ts.then_inc(dve_sem, 1)

        if NP:

            @block.gpsimd
            def _(eng_q):
                eng = nc.gpsimd
                eng.memset(pd0[:, :NP], _SIG_INIT)
                for c in range(N_CHUNK):
                    if c > 0:
                        eng.wait_ge(in_sem, 16 * (NSUB + c))
                    if c >= RINGC:
                        eng.wait_ge(act_sem, c - RINGC + 1)
                    for tl in range(CHUNK):
                        t = c * CHUNK + tl
                        if c == 0 and tl % 2 == 0:
                            eng.wait_ge(in_sem, 16 * (tl // 2 + 1))
                        tau = t % RN
                        sl = t % RING
                        if t == 0:
                            prev = pd0[:, :NP]
                        else:
                            psl = (t - 1) % RING
                            prev = sring[:, psl * H + ND : psl * H + H]
                            if tau == 0:
                                eng.tensor_scalar(
                                    pd0[:, :NP], prev, _RENORM, None,
                                    _Alu.mult, _Alu.bypass,
                                )
                                prev = pd0[:, :NP]
                        wcol = yring[:, sl * H + ND : sl * H + H]
                        ocol = sring[:, sl * H + ND : sl * H + H]
                        eng.tensor_scalar(
                            pd1[:, :NP], prev, 0.0, _DC[tau],
                            _Alu.is_gt, _Alu.mult,
                        )
                        eng.tensor_tensor(pd2[:, :NP], prev, pd1[:, :NP], _Alu.add)
                        tt = eng.tensor_tensor(ocol, pd2[:, :NP], wcol, _Alu.add)
                        if (c == LAST and tl % 2 == 1) or (
                            c < LAST and tl == CHUNK - 1
                        ):
                            tt.then_inc(pool_sem, 1)

        @block.scalar
        def _(scalar):
            for c in range(N_CHUNK - 1):
                scalar.wait_ge(dve_sem, c + 1)
                if NP:
                    scalar.wait_ge(pool_sem, c + 1)
                if c >= RINGC:
                    # spike-ring slot free once chunk c-RINGC DMA'd out
                    scalar.wait_ge(out_sem, 16 * (c - RINGC + 1))
                nc.scalar.activation(
                    kring[:, (c % RINGC) * CB : (c % RINGC + 1) * CB],
                    sring[:, (c % RINGC) * CB : (c % RINGC + 1) * CB],
                    mybir.ActivationFunctionType.Sign,
                    0.0,
                    1.0,
                ).then_inc(act_sem, 1)
            # fine-grained last chunk: extract every 2 steps as they complete
            scalar.wait_ge(out_sem, 16 * (LAST - RINGC + 1))
            for g in range(NSUB):
                scalar.wait_ge(dve_sem, N_CHUNK + g)
                if NP:
                    scalar.wait_ge(pool_sem, N_CHUNK + g)
                roff = (LAST % RINGC) * CB + g * 2 * H
                nc.scalar.activation(
                    kring[:, roff : roff + 2 * H],
                    sring[:, roff : roff + 2 * H],
                    mybir.ActivationFunctionType.Sign,
                    0.0,
                    1.0,
                ).then_inc(act_sem, 1)

    return nc


def kernel(inp: np.ndarray, t: np.ndarray, roll_amount) -> np.ndarray:
    T = float(
        np.clip(np.float32(np.asarray(t).reshape(-1)[0]), np.float32(1.0),
                np.float32(5.0))
    )
    roll = int(np.asarray(roll_amount)) % W

    if "k" not in _cache:
        _cache["k"] = _build()
    nc = _cache["k"]

    scale = np.float64(0.95) / np.float64(np.float32(T))
    K1 = np.float64(0.95) * np.float64(0.95) - np.float64(0.95)  # s-form const
    K2 = K1 - np.float64(0.95)  # DVE carry-form const (BETA^2 - 2 BETA)

    inp = np.asarray(inp, dtype=np.float32).reshape(B, CH, CH)
    x = np.roll(inp, roll, axis=2)  # (B, H, W)
    x = np.ascontiguousarray(x.transpose(0, 2, 1))  # (B, W, H) time-major
    x64 = x.astype(np.float64)
    y = np.empty((B, W, H), dtype=np.float32)
    y[:, :, :ND] = (x64[:, :, :ND] * scale + K2).astype(np.float32)
    if NP:
        bpow = (np.float64(0.95) ** (-(np.arange(W) % RN)))[None, :, None]
        y[:, :, ND:] = (x64[:, :, ND:] * (scale * bpow) + K1 * bpow).astype(
            np.float32
        )

    in_maps = [
        {"y": y[c * PB : (c + 1) * PB].reshape(PB, W * H)} for c in range(N_CORES)
    ]
    res = run_bass_kernel_spmd(nc, in_maps, core_ids=list(range(N_CORES)))

    out = np.empty((B, 1, CH, CH), dtype=np.float32)
    for c in range(N_CORES):
        r = res.results[c]["r"].reshape(PB, W, H)  # (b, w, h) uint8
        out[c * PB : (c + 1) * PB, 0] = (r == 1).transpose(0, 2, 1)
    return out
